# revision 29
# baseline (speedup 1.0000x reference)
"""Trainium2 Bass kernel for BEiT attention block (nn_Beit_9560597201107), v2.

Data-parallel over batch: 64 batches -> 8 NeuronCores x 8 batches each
(4 pairs; token columns pair-packed to N2=394).

Dataflow (per pair g, per batch hb, per head-pair sp):
  qkT[c, n]   = sum_k WT[k,c] xT[k,n] + qbias      (features on partitions;
                fp8e4m3 DoubleRow matmuls at 2 rows/cycle -- weights scaled
                x64 into e4m3 range, unscaled for free via the exp
                activation's 1/64^2 scale.  fp8 is confined to q/k because
                their quantization error is damped by the softmax: measured
                1.44e-2 total vs the 2e-2 gate; fp8 v or proj would add
                ~3e-2 and fail)
  v[m, f]     = sum_k xT[k,m] WT_v[k,f]            (+ ones column 65)
  scT[m, n]   = kT[d,m]^T qT[d,n]                  (heads at partition halves)
  eT[m, n]    = exp(scT) * relE[m, n]              (mul split DVE/gpsimd)
  a[n, i, d|1]= eT[m, n-win]^T [v_i | 1]           (65-col matmuls; col 64 =
                                                    softmax denominators)
  A[n, f]     = a * (1/a[:, 64])                   (DVE recip + broadcast mul)
  cT[c, n]    = PE-transpose(A) via identity       (features back on partitions)
  yT[o, n]    = sum_c pwT[c,o] cT[c,n] + bias

The attention chain (sc -> exp -> attn) has ~2us of cross-engine latency per
step; qkT(g+1), v, and proj(g-1) matmuls are queued as filler units and pumped
between steps so the PE never idles.
"""

import os
from collections import deque

import numpy as np
from ml_dtypes import bfloat16

import concourse.bass as bass
import concourse.bacc as bacc
import concourse.mybir as mybir
import concourse.tile as tile
from concourse.bass_utils import run_bass_kernel_spmd
from concourse.bass_interp import get_hw_module

B, N, DIM, HEADS, NBS = 64, 197, 768, 12, 10
HEAD_DIM = DIM // HEADS
SCALE = HEAD_DIM ** -0.5
NCORES = 8
BPC = B // NCORES          # batches per core
NPAIR = BPC // 2
KT = DIM // 128            # 6 contraction tiles
N2 = 2 * N                 # 394
TOK_TILES = [(0, 128), (128, 69)]

F32 = mybir.dt.float32
BF16 = mybir.dt.bfloat16
F8 = mybir.dt.float8e4
QK_WS = 64.0               # fp8 weight scale for the q/k projections
DR = mybir.MatmulPerfMode.DoubleRow
IDENT = mybir.ActivationFunctionType.Identity
EXP = mybir.ActivationFunctionType.Exp

_CACHE = {}


def _build_module(hw=True):
    nc = bacc.Bacc("TRN2", target_bir_lowering=False, debug=False)

    xt_d = nc.dram_tensor("xt", [NPAIR, 128, KT, N2], BF16, kind="ExternalInput")
    xt8_d = nc.dram_tensor("xt8", [NPAIR, 128, KT, N2], F8, kind="ExternalInput")
    wtq_d = nc.dram_tensor("wtq", [128, 6, 3, 2, 128], F8, kind="ExternalInput")
    wtk_d = nc.dram_tensor("wtk", [128, 6, 3, 2, 128], F8, kind="ExternalInput")
    wtv_d = nc.dram_tensor("wtv", [128, KT, DIM], BF16, kind="ExternalInput")
    pwt_d = nc.dram_tensor("pwt", [128, KT, DIM], BF16, kind="ExternalInput")
    relt_d = nc.dram_tensor("relt", [128, 6, 2, N2], BF16, kind="ExternalInput")
    qbc_d = nc.dram_tensor("qbc", [128, BPC, KT], F32, kind="ExternalInput")
    vpbt_d = nc.dram_tensor("vpbt", [128, KT, BPC], F32, kind="ExternalInput")
    von_d = nc.dram_tensor("von", [128, 12], BF16, kind="ExternalInput")
    idn_d = nc.dram_tensor("idn", [128, 128], BF16, kind="ExternalInput")
    yt_d = nc.dram_tensor("yt", [NPAIR, KT, 128, N2], F32, kind="ExternalOutput")

    with tile.TileContext(nc) as tc:
        with (
            tc.tile_pool(name="const", bufs=1) as constp,
            tc.tile_pool(name="sb_xT", bufs=3) as sb_xT,
            tc.tile_pool(name="sb_qkT", bufs=2) as sb_qkT,
            tc.tile_pool(name="sb_v", bufs=3) as sb_v,
            tc.tile_pool(name="sb_exp", bufs=3) as sb_exp,
            tc.tile_pool(name="sb_A", bufs=2) as sb_A,
            tc.tile_pool(name="sb_rec", bufs=3) as sb_rec,
            tc.tile_pool(name="sb_cT", bufs=3) as sb_cT,
            tc.tile_pool(name="sb_out", bufs=3) as sb_out,
            # "sc" slots are 4KB (2 banks) x2; shared by score tiles and the
            # transpose psum tiles.  "ps" slots are 1 bank x4; shared by
            # qkT/v/proj accumulators and the attention-output tiles.
            tc.tile_pool(name="ps_sc", bufs=2, space="PSUM") as ps_sc,
            tc.tile_pool(name="ps", bufs=4, space="PSUM") as ps,
        ):
            # ---- persistent data, streamed in consumption order ----
            wtq_sb = constp.tile([128, 6, 3, 2, 128], F8)
            wtk_sb = constp.tile([128, 6, 3, 2, 128], F8)

            xT_tiles = {}
            xT8_tiles = {}

            def load_xT(g):
                t_ = sb_xT.tile([128, KT, N2], BF16, tag="xT", name=f"xT_{g}")
                nc.gpsimd.dma_start(out=t_[:], in_=xt_d.ap()[g])
                xT_tiles[g] = t_

            def load_xT8(g, split=False):
                t_ = sb_xT.tile([128, KT, N2], F8, tag="xT8", name=f"xT8_{g}",
                                bufs=2)
                if split:
                    # kp-pair chunks: the first qkT ct starts on chunk 0
                    # via subtile deps instead of waiting for the full tile
                    for kp in range(3):
                        nc.gpsimd.dma_start(
                            out=t_[:, 2 * kp:2 * kp + 2],
                            in_=xt8_d.ap()[g, :, 2 * kp:2 * kp + 2])
                else:
                    nc.gpsimd.dma_start(out=t_[:], in_=xt8_d.ap()[g])
                xT8_tiles[g] = t_

            nc.gpsimd.dma_start(out=wtq_sb[:], in_=wtq_d.ap())
            load_xT8(0, split=True)
            nc.gpsimd.dma_start(out=wtk_sb[:], in_=wtk_d.ap())
            load_xT(0)
            # xT(1)/xT8(1) are issued after the startup block below so the
            # pair-0-critical transfers get the DMA bandwidth to themselves

            # sync queue in need order: qbc gates the first qkT bias
            # copies, relt the first exp; wtv isn't needed until v(0,0)
            qbc_sb = constp.tile([128, BPC, KT], F32)
            nc.sync.dma_start(out=qbc_sb[:], in_=qbc_d.ap())
            relt_sb = constp.tile([128, 6, 2, N2], BF16)
            nc.sync.dma_start(out=relt_sb[:], in_=relt_d.ap())
            wtv_sb = constp.tile([128, KT, DIM], BF16)
            nc.sync.dma_start(out=wtv_sb[:], in_=wtv_d.ap())
            von_sb = constp.tile([128, 12], BF16)
            nc.sync.dma_start(out=von_sb[:], in_=von_d.ap())
            idn_sb = constp.tile([128, 128], BF16)
            nc.sync.dma_start(out=idn_sb[:], in_=idn_d.ap())
            vpbt_sb = constp.tile([128, KT, BPC], F32)
            nc.sync.dma_start(out=vpbt_sb[:], in_=vpbt_d.ap())
            pwt_sb = constp.tile([128, KT, DIM], BF16)
            nc.sync.dma_start(out=pwt_sb[:], in_=pwt_d.ap())

            # ---------------- emission units ----------------
            qkT_tiles = {}

            def ensure_qkT(g):
                if g not in qkT_tiles:
                    t_ = sb_qkT.tile([128, 12, N2 + 59], BF16, tag="qkT",
                                     name=f"qkT_{g}")
                    nc.gpsimd.memset(t_[:, 6:12, N2:N2 + 59], 0)
                    qkT_tiles[g] = t_
                return qkT_tiles[g]

            def emit_qkT_ct(g, ct):
                xT8 = xT8_tiles[g]
                qkT_sb = ensure_qkT(g)
                w = wtq_sb if ct < 6 else wtk_sb
                qp = ps.tile([128, 512], F32, tag="ps", name=f"qp_{g}_{ct}")
                x8v = xT8.rearrange("p (a b) n -> p a b n", a=3)
                for kp in range(3):
                    nc.tensor.matmul(
                        qp[:, 0:N2], w[:, ct % 6, kp], x8v[:, kp],
                        start=(kp == 0), stop=(kp == 2), perf_mode=DR,
                    )
                for hb in range(2):
                    dst = qkT_sb[:, ct, hb * N:(hb + 1) * N]
                    src = qp[:, hb * N:(hb + 1) * N]
                    if ct < 6:
                        qb = qbc_sb[:, 2 * g + hb, ct:ct + 1]
                        if hb == 0:
                            nc.vector.tensor_scalar_add(dst, src, qb)
                        else:
                            nc.scalar.activation(dst, src, IDENT, bias=qb)
                    else:
                        if hb == 0:
                            nc.vector.tensor_copy(dst, src)
                        else:
                            nc.scalar.copy(dst, src)

            v_tiles = {}

            def emit_v_unit(g, hb, t):
                key = (g, hb)
                if key not in v_tiles:
                    v_tiles[key] = sb_v.tile([128, 2, KT, 2, 65], BF16, tag="v",
                                             name=f"v_{g}_{hb}")
                v_sb = v_tiles[key]
                xT_sb = xT_tiles[g]
                off, mt = TOK_TILES[t]
                nc.gpsimd.tensor_copy(
                    v_sb[:, t, :, :, 64:65],
                    von_sb[:, 0:12].rearrange("p (a i o) -> p a i o", i=2, o=1),
                )
                vp = ps.tile([128, 512], F32, tag="ps", name=f"vp_{g}_{hb}_{t}")
                vp2 = ps.tile([128, 512], F32, tag="ps", name=f"vp2_{g}_{hb}_{t}")
                for k in range(KT):
                    xsl = xT_sb[:, k, hb * N + off:hb * N + off + mt]
                    nc.tensor.matmul(
                        vp[0:mt, 0:512], xsl, wtv_sb[:, k, 0:512],
                        start=(k == 0), stop=(k == KT - 1),
                    )
                    nc.tensor.matmul(
                        vp2[0:mt, 0:256], xsl, wtv_sb[:, k, 512:768],
                        start=(k == 0), stop=(k == KT - 1),
                    )
                nc.vector.tensor_copy(
                    v_sb[0:mt, t, 0:4, 0:2, 0:64],
                    vp[0:mt, 0:512].rearrange("p (a i d) -> p a i d", i=2, d=64),
                )
                nc.scalar.copy(
                    v_sb[0:mt, t, 4:6, 0:2, 0:64],
                    vp2[0:mt, 0:256].rearrange("p (a i d) -> p a i d", i=2, d=64),
                )

            sc_tiles = {}

            def emit_sc(g, hb, sp):
                qkT_sb = qkT_tiles[g]
                sc = ps_sc.tile([128, 1024], F32, tag="sc",
                                name=f"sc_{g}_{hb}_{sp}")
                for t, (off, mt) in enumerate(TOK_TILES):
                    nc.tensor.matmul(
                        sc[0:128, t * 256:t * 256 + N],
                        qkT_sb[0:64, 6 + sp, hb * N + off:hb * N + off + 128],
                        qkT_sb[0:64, sp, hb * N:(hb + 1) * N],
                        start=True, stop=True,
                    )
                    nc.tensor.matmul(
                        sc[0:128, 512 + t * 256:512 + t * 256 + N],
                        qkT_sb[64:128, 6 + sp, hb * N + off:hb * N + off + 128],
                        qkT_sb[64:128, sp, hb * N:(hb + 1) * N],
                        start=True, stop=True,
                    )
                sc_tiles[(hb, sp)] = sc

            exp_tiles = {}

            def emit_exp(g, hb, sp):
                sc = sc_tiles.pop((hb, sp))
                expT = sb_exp.tile([128, 2, N2], BF16, tag="expT",
                                   name=f"expT_{g}_{hb}_{sp}")
                nc.scalar.activation(
                    expT[0:128, :, :].rearrange("p t (h n) -> p h t n", h=2),
                    sc[0:128, :].rearrange("p (s x) -> p s x", s=4)[:, :, 0:N],
                    EXP, scale=1.0 / (QK_WS * QK_WS))
                nc.vector.tensor_mul(
                    expT[0:128, 0, :], expT[0:128, 0, :],
                    relt_sb[0:128, sp, 0, :])
                nc.gpsimd.tensor_mul(
                    expT[0:69, 1, :], expT[0:69, 1, :],
                    relt_sb[0:69, sp, 1, :])
                exp_tiles[(hb, sp)] = expT

            a_tiles = {}

            def emit_attn(g, hb, sp, mid=None):
                expT = exp_tiles.pop((hb, sp))
                v_sb = v_tiles[(g, hb)]
                # [n, (w,i), d|1]; padded to one full PSUM bank, j-stride 128
                a = ps.tile([128, 4, 65], F32, tag="ps",
                            padded_shape=[128, 4, 128], name=f"a_{g}_{hb}_{sp}")
                for t, (moff, mt) in enumerate(TOK_TILES):
                    for w, (noff, nw) in enumerate(TOK_TILES):
                        for i in range(2):
                            j = 2 * w + i
                            nc.tensor.matmul(
                                a[0:nw, j, 0:65],
                                expT[0:mt, t, i * N + noff:i * N + noff + nw],
                                v_sb[0:mt, t, sp, i, 0:65],
                                start=(t == 0 and j == 0), stop=(t == 1),
                                skip_group_check=True,
                            )
                    # the t1 half waits on the gpsimd rel-mul, which lags the
                    # DVE one by ~400ns; give the PE filler to chew on
                    if t == 0 and mid is not None:
                        mid()
                a_tiles[(hb, sp)] = a

            A_tiles = {}

            def ensure_A(g, hb):
                if (g, hb) not in A_tiles:
                    A_tiles[(g, hb)] = sb_A.tile([128, 2, DIM], BF16, tag="A",
                                                 name=f"A_{g}_{hb}")
                return A_tiles[(g, hb)]

            def emit_norm(g, hb, sp):
                a = a_tiles.pop((hb, sp))
                A_sb = ensure_A(g, hb)
                rec = sb_rec.tile([128, 4], F32, tag="rec",
                                  name=f"rec_{g}_{hb}_{sp}")
                # w=1 attention tiles only populate partitions 0:69; keep all
                # reads inside written ranges (race-detector clean)
                for w, (noff, nw) in enumerate(TOK_TILES):
                    nc.vector.reciprocal_approx_fast(
                        out=rec[0:nw, 2 * w:2 * w + 2],
                        in_=a[0:nw, 2 * w:2 * w + 2, 64:65])
                    out_ap = A_sb[0:nw, w, sp * 128:(sp + 1) * 128].rearrange(
                        "p (i d) -> p i d", i=2)
                    in0 = a[0:nw, 2 * w:2 * w + 2, 0:64]
                    in1 = rec[0:nw, 2 * w:2 * w + 2] \
                        .unsqueeze(2).broadcast_to([nw, 2, 64])
                    nc.vector.tensor_mul(out_ap, in0, in1)

            cT_tiles = {}

            def ensure_cT(g):
                if g not in cT_tiles:
                    cT_tiles[g] = sb_cT.tile([128, KT, N2], BF16, tag="cT",
                                             name=f"cT_{g}")
                return cT_tiles[g]

            def emit_transposes(g, hb):
                A_sb = A_tiles.pop((g, hb))
                tr = ps_sc.tile([128, 12, 128], BF16, tag="sc",
                                name=f"tr_{g}_{hb}")
                for w, (noff, nw) in enumerate(TOK_TILES):
                    for c in range(KT):
                        idx = w * 6 + c
                        nc.tensor.matmul(
                            tr[0:128, idx, 0:nw],
                            A_sb[0:nw, w, c * 128:(c + 1) * 128],
                            idn_sb[0:nw, 0:nw],
                            start=(idx in (0, 8)), stop=True,
                            is_transpose=True, skip_group_check=True,
                        )
                cT = ensure_cT(g)
                for w, (noff, nw) in enumerate(TOK_TILES):
                    dst = cT[:, :, hb * N + noff:hb * N + noff + nw]
                    src = tr[0:128, w * 6:(w + 1) * 6, 0:nw]
                    if w == 0:
                        nc.scalar.copy(dst, src)
                    else:
                        nc.vector.tensor_copy(dst, src)

            outR = [None]

            def emit_projT(g, o, c0, c1, batch_tail=False):
                cT = cT_tiles[g]
                w = c1 - c0
                prT = ps.tile([128, 512], F32, tag="ps",
                              name=f"prT_{g}_{o}_{c0}")
                for c in range(KT):
                    nc.tensor.matmul(
                        prT[:, 0:w], pwt_sb[:, c, o * 128:(o + 1) * 128],
                        cT[:, c, c0:c1], start=(c == 0), stop=(c == KT - 1),
                    )
                if batch_tail:
                    # the final pair's right halves: one batched DMA instead
                    # of six ~1us serial descriptor gens after the last matmul
                    if outR[0] is None:
                        outR[0] = sb_out.tile([128, KT, N], F32, tag="outR",
                                              bufs=1, name="outR")
                    nc.vector.tensor_scalar_add(
                        outR[0][:, o, :], prT[:, w - N:w],
                        vpbt_sb[:, o, 2 * g + 1:2 * g + 2])
                    # two DMAs of 3 outputs: the first overlaps projs 3-5
                    if o in (2, KT - 1):
                        lo, hi = (0, 3) if o == 2 else (3, KT)
                        nc.sync.dma_start(
                            out=yt_d.ap()[g, lo:hi, :, N:N2]
                            .transpose([1, 0, 2]),
                            in_=outR[0][:, lo:hi])
                    return
                out_sb = sb_out.tile([128, N2], F32, tag="out",
                                     name=f"out_{g}_{o}_{c0}")
                if c0 == 0:
                    nc.scalar.activation(out_sb[:, 0:N], prT[:, 0:N], IDENT,
                                         bias=vpbt_sb[:, o, 2 * g:2 * g + 1])
                if c1 == N2:
                    nc.vector.tensor_scalar_add(
                        out_sb[:, N:N2], prT[:, w - N:w],
                        vpbt_sb[:, o, 2 * g + 1:2 * g + 2])
                nc.sync.dma_start(out=yt_d.ap()[g, o, :, c0:c1],
                                  in_=out_sb[:, c0:c1])

            # ---------------- filler queue ----------------
            filler_q = deque()
            popped = [0]

            def pump(min_cols):
                got = 0
                while filler_q and got < min_cols:
                    cols, th = filler_q.popleft()
                    th()
                    popped[0] += 1
                    got += cols

            def enqueue_qkT(g):
                for ct in range(12):
                    filler_q.append(
                        (1182, lambda g=g, ct=ct: emit_qkT_ct(g, ct)))

            def enqueue_v(g, hb):
                for t in range(2):
                    filler_q.append(
                        (4608, lambda g=g, hb=hb, t=t: emit_v_unit(g, hb, t)))

            def enqueue_proj(g, c0, c1):
                for o in range(KT):
                    filler_q.append(
                        (KT * (c1 - c0),
                         lambda g=g, o=o: emit_projT(g, o, c0, c1)))

            # ---------------- schedule ----------------
            # pair 0 startup: qkT(0) and v(0, hb=0) emitted densely
            for ct in range(12):
                emit_qkT_ct(0, ct)
            load_xT8(1)
            load_xT(1)
            emit_v_unit(0, 0, 0)
            emit_v_unit(0, 0, 1)

            for g in range(NPAIR):
                if g + 2 < NPAIR:
                    load_xT(g + 2)
                    load_xT8(g + 2)
                enqueue_v(g, 1)
                if g >= 1:
                    enqueue_proj(g - 1, 0, N2)
                if g + 1 < NPAIR:
                    enqueue_qkT(g + 1)
                    enqueue_v(g + 1, 0)
                # units up to here must run before pair g+1's first sc reads
                # qkT(g+1) / attn reads v(g+1,0); proj has no deadline and
                # may roll over into the next pair's pumps
                deadline = popped[0] + len(filler_q)
                # NB: denser in-loop filler regresses: the filler units'
                # PSUM->SBUF copies clog the scalar/vector queues that the
                # exp/rel-mul/norm chain ops run on.  ~half the supply in the
                # step loop, the rest in the pair-end drain, measures best.
                supply = sum(c for c, _ in filler_q)
                div = 30 if g == NPAIR - 1 else 24
                pre = int(supply / div * 0.62)
                post = int(supply / div * 0.41)

                if g == 0:
                    emit_sc(g, 0, 0)
                    emit_exp(g, 0, 0)
                # else: chain (g,0,0) was pre-started at the end of pair g-1,
                # so its exp ran during that pair's transposes/flush block
                for hb in range(2):
                    for sp in range(6):
                        if (hb, sp) != (0, 0):
                            emit_exp(g, hb, sp)
                        if sp < 5:
                            emit_sc(g, hb, sp + 1)
                        elif hb == 0:
                            emit_sc(g, 1, 0)
                        pump(pre)
                        emit_attn(g, hb, sp)
                        emit_norm(g, hb, sp)
                        pump(post)
                        if (hb, sp) == (1, 5) and g + 1 < NPAIR:
                            # the next pair's first sc needs every qkT(g+1)
                            # filler emitted first: flush to the deadline,
                            # then pre-start chain (g+1,0,0) so its exp runs
                            # during the transposes/flush tail below
                            while filler_q and popped[0] < deadline:
                                cols, th = filler_q.popleft()
                                th()
                                popped[0] += 1
                            emit_sc(g + 1, 0, 0)
                            emit_exp(g + 1, 0, 0)
                    pump(800)
                    emit_transposes(g, hb)
                    if g == NPAIR - 1 and hb == 0:
                        enqueue_proj(g, 0, N)
                if g == NPAIR - 1:
                    pump(10 ** 9)

            for o in range(KT):
                emit_projT(NPAIR - 1, o, N, N2, batch_tail=True)

    nc.compile()
    if hw:
        nc.m = get_hw_module(nc.m)
    return nc


def _host_prep(x, qkv_weight, q_bias, v_bias, rel_table, proj_weight, proj_bias,
               b_idx, rel_index):
    from ml_dtypes import float8_e4m3
    x = np.asarray(x, dtype=np.float32)
    W = np.asarray(qkv_weight, dtype=np.float32).copy()
    W[:DIM] *= np.float32(SCALE)
    WT = np.ascontiguousarray(W.T)               # [cin, cout]

    def pack_qk(Wslice):
        # [cin, 768] -> [p, ct, kp, sub, c] with cin = kp*256 + sub*128 + p
        return np.ascontiguousarray(
            (Wslice * np.float32(QK_WS)).reshape(3, 2, 128, 6, 128)
            .transpose(2, 3, 0, 1, 4)).astype(float8_e4m3)

    wtq = pack_qk(WT[:, 0:DIM])
    wtk = pack_qk(WT[:, DIM:2 * DIM])
    wtv = np.ascontiguousarray(
        WT[:, 2 * DIM:].reshape(KT, 128, DIM).transpose(1, 0, 2)).astype(bfloat16)
    pwtT = np.asarray(proj_weight, dtype=np.float32).T   # [cin, cout]
    pwt = np.ascontiguousarray(
        pwtT.reshape(KT, 128, DIM).transpose(1, 0, 2)).astype(bfloat16)

    bi = np.asarray(b_idx).astype(np.int64)
    qb_all = (np.asarray(q_bias, dtype=np.float32)[bi]
              * np.float32(SCALE) * np.float32(QK_WS))
    vb_all = np.asarray(v_bias, dtype=np.float32)[bi]
    # softmax rows sum to 1, so attn @ (1 x vb) == 1 x vb; push the v bias
    # through the projection into the proj bias
    pb_all = (np.asarray(proj_bias, dtype=np.float32)[bi]
              + vb_all @ np.asarray(proj_weight, dtype=np.float32).T)

    ridx = np.asarray(rel_index).astype(np.int64)
    relE = np.exp(np.asarray(rel_table, dtype=np.float32)[ridx.reshape(-1)]
                  .reshape(N, N, HEADS))           # [n, m, h]
    relM = relE.transpose(1, 0, 2)                  # [m, n, h]
    relt = np.zeros((128, 6, 2, N2), dtype=np.float32)
    for t, (off, mt) in enumerate(TOK_TILES):
        seg = relM[off:off + mt]                    # [mt, n, h]
        relt[0:mt, :, t, :] = (seg.reshape(mt, N, 6, 2)
                               .transpose(0, 2, 3, 1).reshape(mt, 6, N2))
    relt = relt.astype(bfloat16)
    von = np.ones((128, 12), dtype=bfloat16)
    idn = np.eye(128, dtype=np.float32).astype(bfloat16)

    in_maps = []
    for c in range(NCORES):
        sl = slice(c * BPC, (c + 1) * BPC)
        xs = x[sl]                                  # [8, 197, 768]
        xtf = np.ascontiguousarray(
            xs.reshape(NPAIR, 2, N, DIM).transpose(0, 3, 1, 2)
            .reshape(NPAIR, KT, 128, N2).transpose(0, 2, 1, 3))
        xt = xtf.astype(bfloat16)
        xt8 = xtf.astype(float8_e4m3)
        qbc = np.ascontiguousarray(
            qb_all[sl].reshape(BPC, KT, 128).transpose(2, 0, 1))
        vpbt = np.ascontiguousarray(
            pb_all[sl].reshape(BPC, KT, 128).transpose(2, 1, 0))
        in_maps.append({
            "xt": xt,
            "xt8": xt8,
            "wtq": wtq,
            "wtk": wtk,
            "wtv": wtv,
            "pwt": pwt,
            "relt": relt,
            "qbc": qbc,
            "vpbt": vpbt,
            "von": von,
            "idn": idn,
        })
    return in_maps


def _install_ntff_hook():
    """Provide antenv.axon_hooks (absent from this image) so bass_utils can
    capture NTFF profiles through libaxon_pjrt.so, and keep artifacts local."""
    if _CACHE.get("hook_installed"):
        return
    import sys
    import types
    import ctypes
    import contextlib

    so_path = "/opt/axon/libaxon_pjrt.so"
    lib = ctypes.CDLL(so_path)
    lib.axon_start_nrt_profile.argtypes = [
        ctypes.POINTER(ctypes.c_int64),
        ctypes.c_size_t,
    ]
    lib.axon_start_nrt_profile.restype = ctypes.c_int64
    lib.axon_stop_nrt_profile.argtypes = [ctypes.c_char_p]
    lib.axon_stop_nrt_profile.restype = ctypes.c_int64

    @contextlib.contextmanager
    def _hook(output_dir, device_ids):
        import jax

        jax.devices()
        if device_ids:
            ids = (ctypes.c_int64 * len(device_ids))(*device_ids)
            rc = lib.axon_start_nrt_profile(ids, len(device_ids))
        else:
            rc = lib.axon_start_nrt_profile(None, 0)
        if rc != 0:
            raise RuntimeError(f"axon_start_nrt_profile rc={rc}")
        try:
            yield
        finally:
            n = lib.axon_stop_nrt_profile(str(output_dir).encode())
            print(f"ntff profile: {n} file(s) written to {output_dir}")

    mod = types.ModuleType("antenv.axon_hooks")
    mod.get_axon_ntff_profile_hook = lambda: _hook
    mod.set_axon_ntff_profile_hook = lambda h: None
    sys.modules["antenv.axon_hooks"] = mod

    import concourse.bass_utils as bu

    bu.upload_artifacts = lambda tmpdir: str(tmpdir)
    _CACHE["hook_installed"] = True


def kernel(**inputs):
    if "nc" not in _CACHE:
        _CACHE["nc"] = _build_module()
    nc = _CACHE["nc"]

    in_maps = _host_prep(**inputs)
    trace = os.environ.get("KERNEL_TRACE", "0") == "1"
    tmpdir = None
    if trace:
        _install_ntff_hook()
        tmpdir = os.environ.get("KERNEL_TRACE_DIR") or None
    res = run_bass_kernel_spmd(nc, in_maps, core_ids=list(range(NCORES)),
                               trace=trace, tmpdir=tmpdir)
    if trace:
        _CACHE["last_exec_time_ns"] = res.exec_time_ns
        _CACHE["last_results"] = res

    ys = []
    for c in range(NCORES):
        yt = np.asarray(res.results[c]["yt"])       # [4, 6, 128, 394]
        ys.append(yt.reshape(NPAIR, KT, 128, 2, N)
                  .transpose(0, 3, 4, 1, 2).reshape(BPC, N, DIM))
    return np.ascontiguousarray(np.concatenate(ys, axis=0), dtype=np.float32)


# revision 30
# speedup vs baseline: 1.0021x; 1.0021x over previous
"""Trainium2 Bass kernel for BEiT attention block (nn_Beit_9560597201107), v2.

Data-parallel over batch: 64 batches -> 8 NeuronCores x 8 batches each
(4 pairs; token columns pair-packed to N2=394).

Dataflow (per pair g, per batch hb, per head-pair sp):
  qkT[c, n]   = sum_k WT[k,c] xT[k,n] + qbias      (features on partitions;
                fp8e4m3 DoubleRow matmuls at 2 rows/cycle -- weights scaled
                x64 into e4m3 range, unscaled for free via the exp
                activation's 1/64^2 scale.  fp8 is confined to q/k because
                their quantization error is damped by the softmax: measured
                1.44e-2 total vs the 2e-2 gate; fp8 v or proj would add
                ~3e-2 and fail)
  v[m, f]     = sum_k xT[k,m] WT_v[k,f]            (+ ones column 65)
  scT[m, n]   = kT[d,m]^T qT[d,n]                  (heads at partition halves)
  eT[m, n]    = exp(scT) * relE[m, n]              (mul split DVE/gpsimd)
  a[n, i, d|1]= eT[m, n-win]^T [v_i | 1]           (65-col matmuls; col 64 =
                                                    softmax denominators)
  A[n, f]     = a * (1/a[:, 64])                   (DVE recip + broadcast mul)
  cT[c, n]    = PE-transpose(A) via identity       (features back on partitions)
  yT[o, n]    = sum_c pwT[c,o] cT[c,n] + bias

The attention chain (sc -> exp -> attn) has ~2us of cross-engine latency per
step; qkT(g+1), v, and proj(g-1) matmuls are queued as filler units and pumped
between steps so the PE never idles.
"""

import os
from collections import deque

import numpy as np
from ml_dtypes import bfloat16

import concourse.bass as bass
import concourse.bacc as bacc
import concourse.mybir as mybir
import concourse.tile as tile
from concourse.bass_utils import run_bass_kernel_spmd
from concourse.bass_interp import get_hw_module

B, N, DIM, HEADS, NBS = 64, 197, 768, 12, 10
HEAD_DIM = DIM // HEADS
SCALE = HEAD_DIM ** -0.5
NCORES = 8
BPC = B // NCORES          # batches per core
NPAIR = BPC // 2
KT = DIM // 128            # 6 contraction tiles
N2 = 2 * N                 # 394
TOK_TILES = [(0, 128), (128, 69)]

F32 = mybir.dt.float32
BF16 = mybir.dt.bfloat16
F8 = mybir.dt.float8e4
QK_WS = 64.0               # fp8 weight scale for the q/k projections
DR = mybir.MatmulPerfMode.DoubleRow
IDENT = mybir.ActivationFunctionType.Identity
EXP = mybir.ActivationFunctionType.Exp

_CACHE = {}


def _build_module(hw=True):
    nc = bacc.Bacc("TRN2", target_bir_lowering=False, debug=False)

    xt_d = nc.dram_tensor("xt", [NPAIR, 128, KT, N2], BF16, kind="ExternalInput")
    xt8_d = nc.dram_tensor("xt8", [NPAIR, 128, KT, N2], F8, kind="ExternalInput")
    wtq_d = nc.dram_tensor("wtq", [128, 6, 3, 2, 128], F8, kind="ExternalInput")
    wtk_d = nc.dram_tensor("wtk", [128, 6, 3, 2, 128], F8, kind="ExternalInput")
    wtv_d = nc.dram_tensor("wtv", [128, KT, DIM], BF16, kind="ExternalInput")
    pwt_d = nc.dram_tensor("pwt", [128, KT, DIM], BF16, kind="ExternalInput")
    relt_d = nc.dram_tensor("relt", [128, 6, 2, N2], BF16, kind="ExternalInput")
    qbc_d = nc.dram_tensor("qbc", [128, BPC, KT], F32, kind="ExternalInput")
    vpbt_d = nc.dram_tensor("vpbt", [128, KT, BPC], F32, kind="ExternalInput")
    von_d = nc.dram_tensor("von", [128, 12], BF16, kind="ExternalInput")
    idn_d = nc.dram_tensor("idn", [128, 128], BF16, kind="ExternalInput")
    yt_d = nc.dram_tensor("yt", [NPAIR, KT, 128, N2], F32, kind="ExternalOutput")

    with tile.TileContext(nc) as tc:
        with (
            tc.tile_pool(name="const", bufs=1) as constp,
            tc.tile_pool(name="sb_xT", bufs=3) as sb_xT,
            tc.tile_pool(name="sb_qkT", bufs=2) as sb_qkT,
            tc.tile_pool(name="sb_v", bufs=3) as sb_v,
            tc.tile_pool(name="sb_exp", bufs=3) as sb_exp,
            tc.tile_pool(name="sb_A", bufs=2) as sb_A,
            tc.tile_pool(name="sb_rec", bufs=3) as sb_rec,
            tc.tile_pool(name="sb_cT", bufs=3) as sb_cT,
            tc.tile_pool(name="sb_out", bufs=3) as sb_out,
            # "sc" slots are 4KB (2 banks) x2; shared by score tiles and the
            # transpose psum tiles.  "ps" slots are 1 bank x4; shared by
            # qkT/v/proj accumulators and the attention-output tiles.
            tc.tile_pool(name="ps_sc", bufs=2, space="PSUM") as ps_sc,
            tc.tile_pool(name="ps", bufs=4, space="PSUM") as ps,
        ):
            # ---- persistent data, streamed in consumption order ----
            wtq_sb = constp.tile([128, 6, 3, 2, 128], F8)
            wtk_sb = constp.tile([128, 6, 3, 2, 128], F8)

            xT_tiles = {}
            xT8_tiles = {}

            def load_xT(g):
                t_ = sb_xT.tile([128, KT, N2], BF16, tag="xT", name=f"xT_{g}")
                nc.gpsimd.dma_start(out=t_[:], in_=xt_d.ap()[g])
                xT_tiles[g] = t_

            def load_xT8(g):
                t_ = sb_xT.tile([128, KT, N2], F8, tag="xT8", name=f"xT8_{g}",
                                bufs=2)
                nc.gpsimd.dma_start(out=t_[:], in_=xt8_d.ap()[g])
                xT8_tiles[g] = t_

            nc.gpsimd.dma_start(out=wtq_sb[:], in_=wtq_d.ap())
            load_xT8(0)
            nc.gpsimd.dma_start(out=wtk_sb[:], in_=wtk_d.ap())
            load_xT(0)
            # xT(1)/xT8(1) are issued after the startup block below so the
            # pair-0-critical transfers get the DMA bandwidth to themselves

            # sync queue in need order: qbc gates the first qkT bias
            # copies, relt the first exp; wtv isn't needed until v(0,0)
            qbc_sb = constp.tile([128, BPC, KT], F32)
            nc.sync.dma_start(out=qbc_sb[:], in_=qbc_d.ap())
            relt_sb = constp.tile([128, 6, 2, N2], BF16)
            nc.sync.dma_start(out=relt_sb[:], in_=relt_d.ap())
            wtv_sb = constp.tile([128, KT, DIM], BF16)
            nc.sync.dma_start(out=wtv_sb[:], in_=wtv_d.ap())
            von_sb = constp.tile([128, 12], BF16)
            nc.sync.dma_start(out=von_sb[:], in_=von_d.ap())
            idn_sb = constp.tile([128, 128], BF16)
            nc.sync.dma_start(out=idn_sb[:], in_=idn_d.ap())
            vpbt_sb = constp.tile([128, KT, BPC], F32)
            nc.sync.dma_start(out=vpbt_sb[:], in_=vpbt_d.ap())
            pwt_sb = constp.tile([128, KT, DIM], BF16)
            nc.sync.dma_start(out=pwt_sb[:], in_=pwt_d.ap())

            # ---------------- emission units ----------------
            qkT_tiles = {}

            def ensure_qkT(g):
                if g not in qkT_tiles:
                    t_ = sb_qkT.tile([128, 12, N2 + 59], BF16, tag="qkT",
                                     name=f"qkT_{g}")
                    nc.gpsimd.memset(t_[:, 6:12, N2:N2 + 59], 0)
                    qkT_tiles[g] = t_
                return qkT_tiles[g]

            def emit_qkT_ct(g, ct):
                xT8 = xT8_tiles[g]
                qkT_sb = ensure_qkT(g)
                w = wtq_sb if ct < 6 else wtk_sb
                qp = ps.tile([128, 512], F32, tag="ps", name=f"qp_{g}_{ct}")
                x8v = xT8.rearrange("p (a b) n -> p a b n", a=3)
                for kp in range(3):
                    nc.tensor.matmul(
                        qp[:, 0:N2], w[:, ct % 6, kp], x8v[:, kp],
                        start=(kp == 0), stop=(kp == 2), perf_mode=DR,
                    )
                for hb in range(2):
                    dst = qkT_sb[:, ct, hb * N:(hb + 1) * N]
                    src = qp[:, hb * N:(hb + 1) * N]
                    if ct < 6:
                        qb = qbc_sb[:, 2 * g + hb, ct:ct + 1]
                        if hb == 0:
                            nc.vector.tensor_scalar_add(dst, src, qb)
                        else:
                            nc.scalar.activation(dst, src, IDENT, bias=qb)
                    else:
                        if hb == 0:
                            nc.vector.tensor_copy(dst, src)
                        else:
                            nc.scalar.copy(dst, src)

            v_tiles = {}

            def emit_v_unit(g, hb, t):
                key = (g, hb)
                if key not in v_tiles:
                    v_tiles[key] = sb_v.tile([128, 2, KT, 2, 65], BF16, tag="v",
                                             name=f"v_{g}_{hb}")
                v_sb = v_tiles[key]
                xT_sb = xT_tiles[g]
                off, mt = TOK_TILES[t]
                nc.gpsimd.tensor_copy(
                    v_sb[:, t, :, :, 64:65],
                    von_sb[:, 0:12].rearrange("p (a i o) -> p a i o", i=2, o=1),
                )
                vp = ps.tile([128, 512], F32, tag="ps", name=f"vp_{g}_{hb}_{t}")
                vp2 = ps.tile([128, 512], F32, tag="ps", name=f"vp2_{g}_{hb}_{t}")
                for k in range(KT):
                    xsl = xT_sb[:, k, hb * N + off:hb * N + off + mt]
                    nc.tensor.matmul(
                        vp[0:mt, 0:512], xsl, wtv_sb[:, k, 0:512],
                        start=(k == 0), stop=(k == KT - 1),
                    )
                    nc.tensor.matmul(
                        vp2[0:mt, 0:256], xsl, wtv_sb[:, k, 512:768],
                        start=(k == 0), stop=(k == KT - 1),
                    )
                nc.vector.tensor_copy(
                    v_sb[0:mt, t, 0:4, 0:2, 0:64],
                    vp[0:mt, 0:512].rearrange("p (a i d) -> p a i d", i=2, d=64),
                )
                nc.scalar.copy(
                    v_sb[0:mt, t, 4:6, 0:2, 0:64],
                    vp2[0:mt, 0:256].rearrange("p (a i d) -> p a i d", i=2, d=64),
                )

            sc_tiles = {}

            def emit_sc(g, hb, sp):
                qkT_sb = qkT_tiles[g]
                sc = ps_sc.tile([128, 1024], F32, tag="sc",
                                name=f"sc_{g}_{hb}_{sp}")
                for t, (off, mt) in enumerate(TOK_TILES):
                    nc.tensor.matmul(
                        sc[0:128, t * 256:t * 256 + N],
                        qkT_sb[0:64, 6 + sp, hb * N + off:hb * N + off + 128],
                        qkT_sb[0:64, sp, hb * N:(hb + 1) * N],
                        start=True, stop=True,
                    )
                    nc.tensor.matmul(
                        sc[0:128, 512 + t * 256:512 + t * 256 + N],
                        qkT_sb[64:128, 6 + sp, hb * N + off:hb * N + off + 128],
                        qkT_sb[64:128, sp, hb * N:(hb + 1) * N],
                        start=True, stop=True,
                    )
                sc_tiles[(hb, sp)] = sc

            exp_tiles = {}

            def emit_exp(g, hb, sp):
                sc = sc_tiles.pop((hb, sp))
                expT = sb_exp.tile([128, 2, N2], BF16, tag="expT",
                                   name=f"expT_{g}_{hb}_{sp}")
                nc.scalar.activation(
                    expT[0:128, :, :].rearrange("p t (h n) -> p h t n", h=2),
                    sc[0:128, :].rearrange("p (s x) -> p s x", s=4)[:, :, 0:N],
                    EXP, scale=1.0 / (QK_WS * QK_WS))
                nc.vector.tensor_mul(
                    expT[0:128, 0, :], expT[0:128, 0, :],
                    relt_sb[0:128, sp, 0, :])
                nc.gpsimd.tensor_mul(
                    expT[0:69, 1, :], expT[0:69, 1, :],
                    relt_sb[0:69, sp, 1, :])
                exp_tiles[(hb, sp)] = expT

            a_tiles = {}

            def emit_attn(g, hb, sp, mid=None):
                expT = exp_tiles.pop((hb, sp))
                v_sb = v_tiles[(g, hb)]
                # [n, (w,i), d|1]; padded to one full PSUM bank, j-stride 128
                a = ps.tile([128, 4, 65], F32, tag="ps",
                            padded_shape=[128, 4, 128], name=f"a_{g}_{hb}_{sp}")
                for t, (moff, mt) in enumerate(TOK_TILES):
                    for w, (noff, nw) in enumerate(TOK_TILES):
                        for i in range(2):
                            j = 2 * w + i
                            nc.tensor.matmul(
                                a[0:nw, j, 0:65],
                                expT[0:mt, t, i * N + noff:i * N + noff + nw],
                                v_sb[0:mt, t, sp, i, 0:65],
                                start=(t == 0 and j == 0), stop=(t == 1),
                                skip_group_check=True,
                            )
                    # the t1 half waits on the gpsimd rel-mul, which lags the
                    # DVE one by ~400ns; give the PE filler to chew on
                    if t == 0 and mid is not None:
                        mid()
                a_tiles[(hb, sp)] = a

            A_tiles = {}

            def ensure_A(g, hb):
                if (g, hb) not in A_tiles:
                    A_tiles[(g, hb)] = sb_A.tile([128, 2, DIM], BF16, tag="A",
                                                 name=f"A_{g}_{hb}")
                return A_tiles[(g, hb)]

            def emit_norm(g, hb, sp):
                a = a_tiles.pop((hb, sp))
                A_sb = ensure_A(g, hb)
                rec = sb_rec.tile([128, 4], F32, tag="rec",
                                  name=f"rec_{g}_{hb}_{sp}")
                # w=1 attention tiles only populate partitions 0:69; keep all
                # reads inside written ranges (race-detector clean)
                for w, (noff, nw) in enumerate(TOK_TILES):
                    nc.vector.reciprocal_approx_fast(
                        out=rec[0:nw, 2 * w:2 * w + 2],
                        in_=a[0:nw, 2 * w:2 * w + 2, 64:65])
                    out_ap = A_sb[0:nw, w, sp * 128:(sp + 1) * 128].rearrange(
                        "p (i d) -> p i d", i=2)
                    in0 = a[0:nw, 2 * w:2 * w + 2, 0:64]
                    in1 = rec[0:nw, 2 * w:2 * w + 2] \
                        .unsqueeze(2).broadcast_to([nw, 2, 64])
                    nc.vector.tensor_mul(out_ap, in0, in1)

            cT_tiles = {}

            def ensure_cT(g):
                if g not in cT_tiles:
                    cT_tiles[g] = sb_cT.tile([128, KT, N2], BF16, tag="cT",
                                             name=f"cT_{g}")
                return cT_tiles[g]

            def emit_transposes(g, hb):
                A_sb = A_tiles.pop((g, hb))
                tr = ps_sc.tile([128, 12, 128], BF16, tag="sc",
                                name=f"tr_{g}_{hb}")
                for w, (noff, nw) in enumerate(TOK_TILES):
                    for c in range(KT):
                        idx = w * 6 + c
                        nc.tensor.matmul(
                            tr[0:128, idx, 0:nw],
                            A_sb[0:nw, w, c * 128:(c + 1) * 128],
                            idn_sb[0:nw, 0:nw],
                            start=(idx in (0, 8)), stop=True,
                            is_transpose=True, skip_group_check=True,
                        )
                cT = ensure_cT(g)
                for w, (noff, nw) in enumerate(TOK_TILES):
                    dst = cT[:, :, hb * N + noff:hb * N + noff + nw]
                    src = tr[0:128, w * 6:(w + 1) * 6, 0:nw]
                    if w == 0:
                        nc.scalar.copy(dst, src)
                    else:
                        nc.vector.tensor_copy(dst, src)

            outR = [None]

            def emit_projT(g, o, c0, c1, batch_tail=False):
                cT = cT_tiles[g]
                w = c1 - c0
                prT = ps.tile([128, 512], F32, tag="ps",
                              name=f"prT_{g}_{o}_{c0}")
                for c in range(KT):
                    nc.tensor.matmul(
                        prT[:, 0:w], pwt_sb[:, c, o * 128:(o + 1) * 128],
                        cT[:, c, c0:c1], start=(c == 0), stop=(c == KT - 1),
                    )
                if batch_tail:
                    # the final pair's right halves: one batched DMA instead
                    # of six ~1us serial descriptor gens after the last matmul
                    if outR[0] is None:
                        outR[0] = sb_out.tile([128, KT, N], F32, tag="outR",
                                              bufs=1, name="outR")
                    nc.vector.tensor_scalar_add(
                        outR[0][:, o, :], prT[:, w - N:w],
                        vpbt_sb[:, o, 2 * g + 1:2 * g + 2])
                    # two DMAs of 3 outputs: the first overlaps projs 3-5
                    if o in (2, KT - 1):
                        lo, hi = (0, 3) if o == 2 else (3, KT)
                        nc.sync.dma_start(
                            out=yt_d.ap()[g, lo:hi, :, N:N2]
                            .transpose([1, 0, 2]),
                            in_=outR[0][:, lo:hi])
                    return
                out_sb = sb_out.tile([128, N2], F32, tag="out",
                                     name=f"out_{g}_{o}_{c0}")
                if c0 == 0:
                    nc.scalar.activation(out_sb[:, 0:N], prT[:, 0:N], IDENT,
                                         bias=vpbt_sb[:, o, 2 * g:2 * g + 1])
                if c1 == N2:
                    nc.vector.tensor_scalar_add(
                        out_sb[:, N:N2], prT[:, w - N:w],
                        vpbt_sb[:, o, 2 * g + 1:2 * g + 2])
                nc.sync.dma_start(out=yt_d.ap()[g, o, :, c0:c1],
                                  in_=out_sb[:, c0:c1])

            # ---------------- filler queue ----------------
            filler_q = deque()
            popped = [0]

            def pump(min_cols):
                got = 0
                while filler_q and got < min_cols:
                    cols, th = filler_q.popleft()
                    th()
                    popped[0] += 1
                    got += cols

            def enqueue_qkT(g):
                for ct in range(12):
                    filler_q.append(
                        (1182, lambda g=g, ct=ct: emit_qkT_ct(g, ct)))

            def enqueue_v(g, hb):
                for t in range(2):
                    filler_q.append(
                        (4608, lambda g=g, hb=hb, t=t: emit_v_unit(g, hb, t)))

            def enqueue_proj(g, c0, c1):
                for o in range(KT):
                    filler_q.append(
                        (KT * (c1 - c0),
                         lambda g=g, o=o: emit_projT(g, o, c0, c1)))

            # ---------------- schedule ----------------
            # pair 0 startup: qkT(0) and v(0, hb=0) emitted densely
            for ct in range(12):
                emit_qkT_ct(0, ct)
            load_xT8(1)
            load_xT(1)
            emit_v_unit(0, 0, 0)
            emit_v_unit(0, 0, 1)

            for g in range(NPAIR):
                if g + 2 < NPAIR:
                    load_xT(g + 2)
                    load_xT8(g + 2)
                enqueue_v(g, 1)
                if g >= 1:
                    enqueue_proj(g - 1, 0, N2)
                if g + 1 < NPAIR:
                    enqueue_qkT(g + 1)
                    enqueue_v(g + 1, 0)
                # units up to here must run before pair g+1's first sc reads
                # qkT(g+1) / attn reads v(g+1,0); proj has no deadline and
                # may roll over into the next pair's pumps
                deadline = popped[0] + len(filler_q)
                # NB: denser in-loop filler regresses: the filler units'
                # PSUM->SBUF copies clog the scalar/vector queues that the
                # exp/rel-mul/norm chain ops run on.  ~half the supply in the
                # step loop, the rest in the pair-end drain, measures best.
                supply = sum(c for c, _ in filler_q)
                pre = int(supply / 24 * 0.62)
                post = int(supply / 24 * 0.41)

                if g == 0:
                    emit_sc(g, 0, 0)
                    emit_exp(g, 0, 0)
                # else: chain (g,0,0) was pre-started at the end of pair g-1,
                # so its exp ran during that pair's transposes/flush block
                for hb in range(2):
                    for sp in range(6):
                        if (hb, sp) != (0, 0):
                            emit_exp(g, hb, sp)
                        if sp < 5:
                            emit_sc(g, hb, sp + 1)
                        elif hb == 0:
                            emit_sc(g, 1, 0)
                        pump(pre)
                        emit_attn(g, hb, sp)
                        emit_norm(g, hb, sp)
                        pump(post)
                        if (hb, sp) == (1, 5) and g + 1 < NPAIR:
                            # the next pair's first sc needs every qkT(g+1)
                            # filler emitted first: flush to the deadline,
                            # then pre-start chain (g+1,0,0) so its exp runs
                            # during the transposes/flush tail below
                            while filler_q and popped[0] < deadline:
                                cols, th = filler_q.popleft()
                                th()
                                popped[0] += 1
                            emit_sc(g + 1, 0, 0)
                            emit_exp(g + 1, 0, 0)
                    pump(800)
                    emit_transposes(g, hb)
                    if g == NPAIR - 1 and hb == 0:
                        enqueue_proj(g, 0, N)
                if g == NPAIR - 1:
                    pump(10 ** 9)

            for o in range(KT):
                emit_projT(NPAIR - 1, o, N, N2, batch_tail=True)

    nc.compile()
    if hw:
        nc.m = get_hw_module(nc.m)
    return nc


def _host_prep(x, qkv_weight, q_bias, v_bias, rel_table, proj_weight, proj_bias,
               b_idx, rel_index):
    from ml_dtypes import float8_e4m3
    x = np.asarray(x, dtype=np.float32)
    W = np.asarray(qkv_weight, dtype=np.float32).copy()
    W[:DIM] *= np.float32(SCALE)
    WT = np.ascontiguousarray(W.T)               # [cin, cout]

    def pack_qk(Wslice):
        # [cin, 768] -> [p, ct, kp, sub, c] with cin = kp*256 + sub*128 + p
        return np.ascontiguousarray(
            (Wslice * np.float32(QK_WS)).reshape(3, 2, 128, 6, 128)
            .transpose(2, 3, 0, 1, 4)).astype(float8_e4m3)

    wtq = pack_qk(WT[:, 0:DIM])
    wtk = pack_qk(WT[:, DIM:2 * DIM])
    wtv = np.ascontiguousarray(
        WT[:, 2 * DIM:].reshape(KT, 128, DIM).transpose(1, 0, 2)).astype(bfloat16)
    pwtT = np.asarray(proj_weight, dtype=np.float32).T   # [cin, cout]
    pwt = np.ascontiguousarray(
        pwtT.reshape(KT, 128, DIM).transpose(1, 0, 2)).astype(bfloat16)

    bi = np.asarray(b_idx).astype(np.int64)
    qb_all = (np.asarray(q_bias, dtype=np.float32)[bi]
              * np.float32(SCALE) * np.float32(QK_WS))
    vb_all = np.asarray(v_bias, dtype=np.float32)[bi]
    # softmax rows sum to 1, so attn @ (1 x vb) == 1 x vb; push the v bias
    # through the projection into the proj bias
    pb_all = (np.asarray(proj_bias, dtype=np.float32)[bi]
              + vb_all @ np.asarray(proj_weight, dtype=np.float32).T)

    ridx = np.asarray(rel_index).astype(np.int64)
    relE = np.exp(np.asarray(rel_table, dtype=np.float32)[ridx.reshape(-1)]
                  .reshape(N, N, HEADS))           # [n, m, h]
    relM = relE.transpose(1, 0, 2)                  # [m, n, h]
    relt = np.zeros((128, 6, 2, N2), dtype=np.float32)
    for t, (off, mt) in enumerate(TOK_TILES):
        seg = relM[off:off + mt]                    # [mt, n, h]
        relt[0:mt, :, t, :] = (seg.reshape(mt, N, 6, 2)
                               .transpose(0, 2, 3, 1).reshape(mt, 6, N2))
    relt = relt.astype(bfloat16)
    von = np.ones((128, 12), dtype=bfloat16)
    idn = np.eye(128, dtype=np.float32).astype(bfloat16)

    in_maps = []
    for c in range(NCORES):
        sl = slice(c * BPC, (c + 1) * BPC)
        xs = x[sl]                                  # [8, 197, 768]
        xtf = np.ascontiguousarray(
            xs.reshape(NPAIR, 2, N, DIM).transpose(0, 3, 1, 2)
            .reshape(NPAIR, KT, 128, N2).transpose(0, 2, 1, 3))
        xt = xtf.astype(bfloat16)
        xt8 = xtf.astype(float8_e4m3)
        qbc = np.ascontiguousarray(
            qb_all[sl].reshape(BPC, KT, 128).transpose(2, 0, 1))
        vpbt = np.ascontiguousarray(
            pb_all[sl].reshape(BPC, KT, 128).transpose(2, 1, 0))
        in_maps.append({
            "xt": xt,
            "xt8": xt8,
            "wtq": wtq,
            "wtk": wtk,
            "wtv": wtv,
            "pwt": pwt,
            "relt": relt,
            "qbc": qbc,
            "vpbt": vpbt,
            "von": von,
            "idn": idn,
        })
    return in_maps


def _install_ntff_hook():
    """Provide antenv.axon_hooks (absent from this image) so bass_utils can
    capture NTFF profiles through libaxon_pjrt.so, and keep artifacts local."""
    if _CACHE.get("hook_installed"):
        return
    import sys
    import types
    import ctypes
    import contextlib

    so_path = "/opt/axon/libaxon_pjrt.so"
    lib = ctypes.CDLL(so_path)
    lib.axon_start_nrt_profile.argtypes = [
        ctypes.POINTER(ctypes.c_int64),
        ctypes.c_size_t,
    ]
    lib.axon_start_nrt_profile.restype = ctypes.c_int64
    lib.axon_stop_nrt_profile.argtypes = [ctypes.c_char_p]
    lib.axon_stop_nrt_profile.restype = ctypes.c_int64

    @contextlib.contextmanager
    def _hook(output_dir, device_ids):
        import jax

        jax.devices()
        if device_ids:
            ids = (ctypes.c_int64 * len(device_ids))(*device_ids)
            rc = lib.axon_start_nrt_profile(ids, len(device_ids))
        else:
            rc = lib.axon_start_nrt_profile(None, 0)
        if rc != 0:
            raise RuntimeError(f"axon_start_nrt_profile rc={rc}")
        try:
            yield
        finally:
            n = lib.axon_stop_nrt_profile(str(output_dir).encode())
            print(f"ntff profile: {n} file(s) written to {output_dir}")

    mod = types.ModuleType("antenv.axon_hooks")
    mod.get_axon_ntff_profile_hook = lambda: _hook
    mod.set_axon_ntff_profile_hook = lambda h: None
    sys.modules["antenv.axon_hooks"] = mod

    import concourse.bass_utils as bu

    bu.upload_artifacts = lambda tmpdir: str(tmpdir)
    _CACHE["hook_installed"] = True


def kernel(**inputs):
    if "nc" not in _CACHE:
        _CACHE["nc"] = _build_module()
    nc = _CACHE["nc"]

    in_maps = _host_prep(**inputs)
    trace = os.environ.get("KERNEL_TRACE", "0") == "1"
    tmpdir = None
    if trace:
        _install_ntff_hook()
        tmpdir = os.environ.get("KERNEL_TRACE_DIR") or None
    res = run_bass_kernel_spmd(nc, in_maps, core_ids=list(range(NCORES)),
                               trace=trace, tmpdir=tmpdir)
    if trace:
        _CACHE["last_exec_time_ns"] = res.exec_time_ns
        _CACHE["last_results"] = res

    ys = []
    for c in range(NCORES):
        yt = np.asarray(res.results[c]["yt"])       # [4, 6, 128, 394]
        ys.append(yt.reshape(NPAIR, KT, 128, 2, N)
                  .transpose(0, 3, 4, 1, 2).reshape(BPC, N, DIM))
    return np.ascontiguousarray(np.concatenate(ys, axis=0), dtype=np.float32)
